# revision 10
# baseline (speedup 1.0000x reference)
"""Trainium2 Bass kernel for nn_ChannelGroupConvUneven.

Computes, for full inputs
    x      (8, 256, 128, 128) f32
    weight (320, 256, 3, 3)   f32
    bias   (320,)             f32
    param  (5,)               i32   per-group input-channel thresholds
the reference
    out = conv2d(x, weight * mask(param), stride 1, VALID) + bias
    out shape (8, 320, 126, 126) f32
where mask zeroes weight[o, i] for i < param[o // 64].

Strategy: data-parallel over batch — one image per NeuronCore (8 cores),
weights replicated. Weight masking happens on the host (tiny, exact for any
runtime `param`).

Algorithm: 1D Winograd F(4,3) along the width axis, direct convolution along
height. Per 4-wide output tile the width conv needs 6 winograd-domain
products instead of 12 direct MACs, so the PE does 6 taps x (2 cin blocks x
3 dy) = 36 matmuls per 4 output rows instead of direct conv's 72 — half the
moving rows, which is what bounds the PE (bf16/fp16 stream 1 moving row per
cycle regardless of dtype; fp8's DoubleRow 2x mode fails the accuracy gate:
measured rel err 3.2e-2 > 2e-2).

The input transform V = B^T d (per cin, per row, 32 width-tiles of 6 taps)
and the weight transform U = G w are computed ON THE HOST in f32 and shipped
as fp16 (fp16 V/U measured end-to-end rel err 2.7e-3 vs bf16's 1.1e-2; same
DMA bytes as bf16). The device only runs the winograd-domain GEMMs and the
light output transform A^T m (10 DVE ops per quad, scalings folded via
scalar_tensor_tensor). Bias is added on the host after the f32 upconvert.

Geometry: 126 output cols = 32 tiles of 4 at stride 4, the last tile shifted
to start at col 122 (cols 122/123 computed twice, identical values). Same
trick for rows: 32 quads of 4 rows, last quad at rows 122..125. M = 4 rows x
32 tiles = 128 — full PE width. N = 320 couts in one PSUM bank (1280 B).

PSUM: 6 banks per quad (one per winograd tap t), filled in order t1,t2,t3,
t4,t0,t5 so the DVE combine (s=m1+m2, d=m1-m2, e=m3+m4, f=m3-m4) can start
mid-quad and release banks for the next quad; 8-bank pool double-buffers.
Output rows are written as [quad, j, 4row x 32tile, cout] bf16 and
de-interleaved on the host.
"""

import numpy as np

import concourse.mybir as mybir
import concourse.tile as tile
from concourse import bacc
from concourse.bass_utils import run_bass_kernel_spmd


def _ensure_axon_ntff_hook():
    """Best-effort: register the axon NTFF profile hook if the image's
    `antenv` stub lacks `axon_hooks` (concourse's trace path imports it
    unconditionally when BASS_TRACE is set). Purely optional — failures are
    ignored and tracing is simply unavailable."""
    try:
        import sys
        import types

        import antenv

        if "antenv.axon_hooks" in sys.modules:
            return
        mod = types.ModuleType("antenv.axon_hooks")
        _hook = [None]
        mod.set_axon_ntff_profile_hook = lambda h: _hook.__setitem__(0, h)
        mod.get_axon_ntff_profile_hook = lambda: _hook[0]
        sys.modules["antenv.axon_hooks"] = mod
        antenv.axon_hooks = mod
        from trn_agent_boot.trn_boot import _ntff_profile_via_ctypes

        mod.set_axon_ntff_profile_hook(
            _ntff_profile_via_ctypes("/opt/axon/libaxon_pjrt.so")
        )
    except Exception:
        pass


_ensure_axon_ntff_hook()

N_CORES = 8
P = 128
CIN, COUT, KH, KW = 256, 320, 3, 3
H = W = 128
HO = WO = 126
CB = CIN // P  # 2 cin blocks

# F(4,3) winograd, interpolation points 0, 1, -1, 2, -2, inf
NT = 6  # winograd taps per tile
TM = 4  # outputs per tile
NTILE = 32  # width tiles per row (last one shifted to start at col 122)
NQUAD = 32  # row quads (last one shifted to start at row 122)
# tile/quad start offsets: 0,4,...,120,122
_STARTS = [min(TM * i, W - NT) for i in range(NTILE)]

# B^T rows: coefficients of M_i(x) = prod_{j != i}(x - a_j), ascending powers
_BT = np.array(
    [
        [4, 0, -5, 0, 1, 0],
        [0, -4, -4, 1, 1, 0],
        [0, 4, -4, -1, 1, 0],
        [0, -2, -1, 2, 1, 0],
        [0, 2, -1, -2, 1, 0],
        [0, 4, 0, -5, 0, 1],
    ],
    dtype=np.float64,
)
# G rows: [1, a, a^2] / prod_{j != i}(a_i - a_j), last row e_2
_G = np.array(
    [
        [1 / 4, 0, 0],
        [-1 / 6, -1 / 6, -1 / 6],
        [-1 / 6, 1 / 6, -1 / 6],
        [1 / 24, 2 / 24, 4 / 24],
        [1 / 24, -2 / 24, 4 / 24],
        [0, 0, 1],
    ],
    dtype=np.float64,
)
# A^T[j, t] = a_t^j (used implicitly by the on-device output transform):
#   y0 = m0 + m1 + m2 + m3 + m4
#   y1 =      m1 - m2 + 2(m3 - m4)
#   y2 =      m1 + m2 + 4(m3 + m4)
#   y3 =      m1 - m2 + 8(m3 - m4) + m5

_NC_CACHE = {}


def _build_nc():
    nc = bacc.Bacc("TRN2", target_bir_lowering=False, debug=False)
    f32 = mybir.dt.float32
    f16 = mybir.dt.float16
    bf16 = mybir.dt.bfloat16

    v_d = nc.dram_tensor(
        "v", [P, CB, NT, H, NTILE], f16, kind="ExternalInput"
    ).ap()
    u_d = nc.dram_tensor(
        "u", [P, CB, KH, NT, COUT], f16, kind="ExternalInput"
    ).ap()
    o_d = nc.dram_tensor(
        "out", [NQUAD, TM, P, COUT], bf16, kind="ExternalOutput"
    ).ap()

    add = mybir.AluOpType.add
    sub = mybir.AluOpType.subtract
    mult = mybir.AluOpType.mult

    # matmul group order within a quad: t1..t4 first so the DVE can start
    # combining (and freeing banks) while t0/t5 stream
    T_ORDER = [1, 2, 3, 4, 0, 5]

    with tile.TileContext(nc) as tc:
        with (
            tc.tile_pool(name="wpool", bufs=1) as wpool,
            tc.tile_pool(name="tmp", bufs=2) as tmp_pool,
            tc.tile_pool(name="ypool", bufs=8) as ypool,
            tc.tile_pool(name="psum", bufs=8, space="PSUM") as psum_pool,
        ):
            vt = wpool.tile([P, CB, NT, H, NTILE], f16)
            ut = wpool.tile([P, CB, KH, NT, COUT], f16)

            # DMA staging: U taps and the first V rows in mm order across the
            # sync + scalar HWDGE queues (each drains in program order, so
            # first-needed goes first; gpsimd is kept free for its share of
            # the output transform). V h-chunks are small (16 rows, 1KB per
            # partition) only where needed early; later chunks are 32 rows
            # (2KB per partition) for better DMA descriptor efficiency.
            queues = [nc.sync, nc.scalar]
            dmas = []
            for t in T_ORDER:
                for cb in range(CB):
                    dmas.append((ut[:, cb, :, t], u_d[:, cb, :, t]))
                    dmas.append((vt[:, cb, t, 0:16], v_d[:, cb, t, 0:16]))
            for h0, h1 in ((16, 48), (48, 80), (80, 112), (112, 128)):
                for t in T_ORDER:
                    for cb in range(CB):
                        dmas.append(
                            (
                                vt[:, cb, t, h0:h1],
                                v_d[:, cb, t, h0:h1],
                            )
                        )
            for i, (dst, src) in enumerate(dmas):
                queues[i % 2].dma_start(dst, src)

            out_q = [nc.sync, nc.scalar]
            for q in range(NQUAD):
                qs = min(TM * q, W - NT)
                ps = {}
                for t in T_ORDER:
                    ps[t] = psum_pool.tile(
                        [P, COUT], f32, tag="ps", name=f"ps_q{q}_t{t}"
                    )
                    k = 0
                    for cb in range(CB):
                        for dy in range(KH):
                            nc.tensor.matmul(
                                ps[t][:],
                                vt[:, cb, t, qs + dy : qs + dy + TM, :],
                                ut[:, cb, dy, t, :],
                                start=(k == 0),
                                stop=(k == CB * KH - 1),
                            )
                            k += 1

                # output transform: y_j = A^T m, scalings fused via
                # scalar_tensor_tensor; intermediates in f32 SBUF. A DVE op
                # may read at most ONE PSUM operand (NCC_IBVF027), so m1/m3
                # are first evacuated by the scalar engine, which also frees
                # their banks early for the next quad.
                c0 = tmp_pool.tile([P, COUT], f32, tag="c0")
                c1 = tmp_pool.tile([P, COUT], f32, tag="c1")
                c3 = tmp_pool.tile([P, COUT], f32, tag="c3")
                c5 = tmp_pool.tile([P, COUT], f32, tag="c5")
                s = tmp_pool.tile([P, COUT], f32, tag="s")
                d = tmp_pool.tile([P, COUT], f32, tag="d")
                e = tmp_pool.tile([P, COUT], f32, tag="e")
                f = tmp_pool.tile([P, COUT], f32, tag="f")
                ta = tmp_pool.tile([P, COUT], f32, tag="ta")
                tb = tmp_pool.tile([P, COUT], f32, tag="tb")
                y = [
                    ypool.tile([P, COUT], bf16, tag="y", name=f"y_q{q}_{j}")
                    for j in range(4)
                ]

                # GpSimd cannot access PSUM, so: ACT evacuates the four
                # single-use banks (freeing each right after its group), DVE
                # does the four PSUM-reading combines, GpSimd the SBUF-only
                # rest. Each engine stays well under the PE's ~5us per quad.
                nc.scalar.copy(c1[:], ps[1][:])
                nc.scalar.copy(c3[:], ps[3][:])
                nc.vector.tensor_tensor(s[:], c1[:], ps[2][:], add)
                nc.vector.tensor_tensor(d[:], c1[:], ps[2][:], sub)
                nc.vector.tensor_tensor(e[:], c3[:], ps[4][:], add)
                nc.vector.tensor_tensor(f[:], c3[:], ps[4][:], sub)
                nc.scalar.copy(c0[:], ps[0][:])
                nc.scalar.copy(c5[:], ps[5][:])
                # scalar_tensor_tensor doesn't lower on Pool, so the fused
                # scale+add ops stay on DVE; GpSimd takes plain adds only
                nc.gpsimd.tensor_tensor(ta[:], c0[:], s[:], add)
                nc.gpsimd.tensor_tensor(y[0][:], ta[:], e[:], add)
                nc.vector.scalar_tensor_tensor(
                    y[1][:], f[:], 2.0, d[:], mult, add
                )
                nc.vector.scalar_tensor_tensor(
                    y[2][:], e[:], 4.0, s[:], mult, add
                )
                nc.vector.scalar_tensor_tensor(
                    tb[:], f[:], 8.0, d[:], mult, add
                )
                nc.gpsimd.tensor_tensor(y[3][:], tb[:], c5[:], add)
                for j in range(4):
                    out_q[(4 * q + j) % 2].dma_start(o_d[q, j], y[j][:])
    nc.compile()
    return nc


def _get_nc():
    key = "wino43-f16-v1"
    if key not in _NC_CACHE:
        _NC_CACHE[key] = _build_nc()
    return _NC_CACHE[key]


def _preprocess(x, weight, bias, param):
    x = np.asarray(x, dtype=np.float32)
    weight = np.asarray(weight, dtype=np.float32)
    bias = np.asarray(bias, dtype=np.float32)
    param = np.asarray(param)

    # host-side weight masking (group g of 64 output channels uses cin >=
    # param[g]); mask before the winograd transform / fp16 round
    thresh = np.repeat(param.astype(np.int64), COUT // param.shape[0])
    mask = (np.arange(CIN)[None, :] >= thresh[:, None]).astype(np.float32)
    wm = (weight * mask[:, :, None, None]).astype(np.float64)

    # weight transform U[t, dy, o, c] = sum_r G[t, r] * w[o, c, dy, r],
    # device layout [p, cb, dy, t, cout] fp16
    U = np.einsum("tr,ocyr->ycot", _G, wm)  # [dy, c, o, t]
    U = U.reshape(KH, CB, P, COUT, NT).transpose(2, 1, 0, 4, 3)
    U = np.ascontiguousarray(U).astype(np.float16)

    # input transform V[b, c, t, h, k] = sum_i BT[t, i] * x[b, c, h, sk + i],
    # device layout per core [p, cb, t, h, 32] fp16
    starts = np.asarray(_STARTS)
    gath = x[:, :, :, starts[:, None] + np.arange(NT)[None, :]]
    # gath: [8, 256, 128, 32, 6] @ BT.T -> [8, 256, 128, 32, 6(t)]
    V = gath.reshape(-1, NT) @ _BT.T.astype(np.float32)
    V = V.reshape(N_CORES, CB, P, H, NTILE, NT).transpose(0, 2, 1, 5, 3, 4)
    V = np.ascontiguousarray(V).astype(np.float16)
    return V, U


def _postprocess(results, bias):
    # per-core "out" is [quad, j, 4row x 32tile, cout] bf16; map to
    # [cout, row, col] f32: row = qstart[q] + rr, col = tstart[k] + j
    # (overlapping quad/tile 31 rewrite rows/cols 122-123 with identical
    # values)
    arr = np.stack([np.asarray(r["out"]) for r in results], axis=0)
    arr = arr.astype(np.float32)  # [8, 32q, 4j, 128(rr*32+k), 320]
    arr = arr.reshape(N_CORES, NQUAD, TM, TM, NTILE, COUT)
    starts = np.asarray(_STARTS)
    rows = (starts[:, None] + np.arange(TM)[None, :])  # [q, rr]
    cols = (starts[:, None] + np.arange(TM)[None, :])  # [k, j]
    out = np.empty((N_CORES, COUT, HO, WO), dtype=np.float32)
    # arr indexed [b, q, j, rr, k, o] -> out[b, o, rows[q, rr], cols[k, j]]
    out[
        :,
        :,
        rows[:, None, :, None],  # [q, 1, rr, 1]
        cols[None, :, None, :],  # [1, k, 1, j]
    ] = arr.transpose(0, 5, 1, 4, 3, 2)  # [b, o, q, k, rr, j]
    out += bias[None, :, None, None]
    return out


def _make_in_maps(x, weight, bias, param):
    V, U = _preprocess(x, weight, bias, param)
    return [{"v": V[i], "u": U} for i in range(N_CORES)]


def kernel(x, weight, bias, param):
    bias = np.asarray(bias, dtype=np.float32)
    in_maps = _make_in_maps(x, weight, bias, param)
    nc = _get_nc()
    res = run_bass_kernel_spmd(nc, in_maps, core_ids=list(range(N_CORES)))
    return _postprocess(res.results, bias)
